# revision 56
# baseline (speedup 1.0000x reference)
"""Trainium2 Bass kernel for nn_MultiHeadAttention (B=2, S=2048, D=1024, H=16).

Sharding: batch*heads across 8 cores -> each core handles one batch element's
4 heads (core c: b = c//4, heads h0 = (c%4)*4 .. h0+4).

Key idea: the padding mask kills ~half the keys; the host gathers each head's
unmasked key positions (padded to KT tiles of 128) so scores/exp/ctx run over
~9 instead of 16 key tiles.

v2 restructure (from trace analysis of the ~165us v1):
  - The PE queue is IN-ORDER, and v1 emitted ALL of Q/K/V projection before
    the first score matmul, so the first softmax exp ran at t=65us (gated by
    the last xg DMA + KV h3 on the PE). v2 starts attention right after
    KV h0,h1 and INJECTS the remaining projection work (Qproj for the second
    query half, KV h2,h3, VT transposes) into the early attention steps,
    borrowing the score ring's PSUM slots (tag "a" only: the ctx ring
    turns over once per block and would stall the in-order PE queue).
  - Dead start: v1's first DMA packet landed at 8.7us (consts queued ahead
    on the sync engine) and the PE warmup waited on big gpsimd memsets.
    v2 issues const DMAs from the (idle) scalar queue, streams the big
    inputs on sync in first-use order, and feeds the warmup from a tiny
    ones4k DMA so the PE p-state ramp starts at ~2us.
  - VT bias-drains of the injected heads run on DVE, not ScalarE
    (ScalarE is the exp engine; each stolen slot delays the exp stream;
    gpsimd cannot read PSUM).
  - Tail: v1 had a 7.2us all-engine gap after the last block's
    softmax-sum DMA-transpose chain, after which the PE clock had dropped
    to 1.2GHz for the entire second half of the output projection. v2
    keeps the PE grinding filler matmuls through the chain (clock stays
    at 2.4GHz) and batches the output DMAs in qt-pairs issued from the
    gpsimd queue (the sync queue serializes DMA issue at ~0.6us each).
  - ctxT is split per (half, pair) so output-projection reads never
    coarse-dep on the last block's writes.
Host sums the 4 partial outputs per batch element and adds b_out.
"""

import math
import os

import numpy as np

# Tile's fine-grained (subtile) dependency tracker misses some of this
# kernel's partition-sliced producer->consumer edges (verified empirically:
# per-core divergent results with it on, bit-identical and correct with it
# off). Coarse tile-level deps cost little here and are always safe.
os.environ.setdefault("BY_DEFAULT_DISABLE_SUBTILE_DEPS", "1")

N_HEADS = 16
DIM = 1024
DIM_PER_HEAD = 64
B = 2
S = 2048
SCALE = math.sqrt(DIM_PER_HEAD)
N_CORES = 8
HEADS_PER_CORE = 4

_cache = {}


def _build_program(KT):
    import concourse.tile as tile
    from concourse import bacc, mybir

    f32 = mybir.dt.float32
    fp16 = mybir.dt.float16
    Exp = mybir.ActivationFunctionType.Exp
    SK = KT * 128  # gathered (padded) key count per head

    nc = bacc.Bacc("TRN2", target_bir_lowering=False, debug=False,
                   num_devices=N_CORES)

    # All big inputs are pre-laid-out on the HOST in partition-major tile
    # form: a strided rearrange here costs the DMA queue 3-7us of
    # descriptor generation per tensor (1024 descriptors), which was
    # measured to stretch the 15MB input stream from ~390 to ~260GB/s
    # effective. Contiguous-per-partition sources need ~128 descriptors.
    xT = nc.dram_tensor("xT", [4, 128, 8, 512], fp16,
                        kind="ExternalInput").ap()
    xg = nc.dram_tensor("xg", [4, 128, 8, SK], fp16,
                        kind="ExternalInput").ap()
    Wq = nc.dram_tensor("Wq", [128, 8, 256], fp16,
                        kind="ExternalInput").ap()
    Wkv = nc.dram_tensor("Wkv", [128, 8, 512], fp16,
                         kind="ExternalInput").ap()
    Wo = nc.dram_tensor("Wo", [128, 2, 1024], fp16,
                        kind="ExternalInput").ap()
    bqk = nc.dram_tensor("bqk", [128, 4], f32, kind="ExternalInput").ap()
    bvT = nc.dram_tensor("bvT", [128, 4], f32, kind="ExternalInput").ap()
    id2 = nc.dram_tensor("id2", [128, 64], fp16, kind="ExternalInput").ap()
    maskT = nc.dram_tensor("maskT", [128, 4 * KT], f32,
                           kind="ExternalInput").ap()
    out_d = nc.dram_tensor("out", [S, DIM], fp16, kind="ExternalOutput").ap()

    with tile.TileContext(nc) as tc:
        with tc.tile_pool(name="const", bufs=1) as cpool, \
             tc.tile_pool(name="wpool", bufs=1) as wpool, \
             tc.tile_pool(name="xgp", bufs=1) as xgp, \
             tc.tile_pool(name="qkv", bufs=1) as qkvp, \
             tc.tile_pool(name="xsub", bufs=1) as xsub, \
             tc.tile_pool(name="ps", bufs=2, space="PSUM") as ps:

            # ---- const DMAs on the SCALAR queue (idle until the exps),
            # so the sync queue starts streaming the big inputs at t~0 ----
            maskT_sb = cpool.tile([128, 4 * KT], f32)
            nc.scalar.dma_start(maskT_sb[:], maskT[:])
            bqk_sb = cpool.tile([128, 4], f32)
            nc.scalar.dma_start(bqk_sb[:], bqk[:])
            bvT_sb = cpool.tile([128, 4], f32)
            nc.scalar.dma_start(bvT_sb[:], bvT[:])
            id_sb = cpool.tile([128, 64], fp16)
            nc.scalar.dma_start(id_sb[:], id2[:])

            # ones row via DVE memset (not DMA): ready the moment the
            # framework preamble ends, so the PE p-state warmup starts
            # ~2us earlier and Qproj runs at full clock
            ones_sb = cpool.tile([1, 512], fp16)
            nc.vector.memset(ones_sb[:], 1.0)
            Wq_sb = wpool.tile([128, 8, 256], fp16)
            nc.sync.dma_start(Wq_sb[:], Wq[:])

            # Q targets, per (row-half, query-half). half h covers query
            # columns h*1024..h*1024+1024; zero halves memset on gpsimd.
            # Separate half-tiles let the injected Qproj of half 1 avoid
            # coarse-dep serialization against running scores on half 0.
            Qt0_h = [qkvp.tile([128, 2, 1024], fp16, name=f"Qt0_{h}")
                     for h in range(2)]
            Qt1_h = [qkvp.tile([128, 2, 1024], fp16, name=f"Qt1_{h}")
                     for h in range(2)]
            for h in range(2):
                nc.gpsimd.memset(Qt1_h[h][0:64, :, :], 0.0)
                nc.gpsimd.memset(Qt0_h[h][64:128, :, :], 0.0)
            Kt_p = [qkvp.tile([128, SK], fp16, name=f"Kt_{p}")
                    for p in range(2)]
            v4_h = [qkvp.tile([128, KT, 65], fp16, name=f"v4_{hl}")
                    for hl in range(4)]
            for hl in range(4):
                nc.gpsimd.memset(v4_h[hl][:, :, 64], 1.0)
            # ctxT per (half, pair): po reads never wait on the other pair
            ctxT_hp = [[qkvp.tile([128, 1024], fp16, name=f"ctxT_{h}_{p}")
                        for p in range(2)] for h in range(2)]

            # big-input stream order = PE first-use order: the projections
            # are emitted serially (the in-order PE queue runs them in
            # emission order), so each tensor lands just before its
            # consumer while the PE grinds the previous one.
            xts = [None] * 4
            for sc in (0, 1):
                t = xsub.tile([128, 8, 512], fp16, name=f"xts_{sc}")
                nc.sync.dma_start(t[:], xT[sc])
                xts[sc] = t
            Wkv_sb = wpool.tile([128, 8, 512], fp16)
            nc.sync.dma_start(Wkv_sb[:], Wkv[:])
            xg_t = [None] * 4
            t = xgp.tile([128, 8, SK], fp16, name="xg_0")
            nc.sync.dma_start(t[:], xg[0])
            xg_t[0] = t
            for sc in (2, 3):
                t = xsub.tile([128, 8, 512], fp16, name=f"xts_{sc}")
                nc.sync.dma_start(t[:], xT[sc])
                xts[sc] = t
            for hl in (1, 2, 3):
                t = xgp.tile([128, 8, SK], fp16, name=f"xg_{hl}")
                nc.sync.dma_start(t[:], xg[hl])
                xg_t[hl] = t
            Wo_sb = wpool.tile([128, 2, 1024], fp16)
            nc.sync.dma_start(Wo_sb[:], Wo[:])

            # ---- PE p-state warmup: TRN2's tensor engine needs ~3us of
            # continuous work to reach 2.4GHz (idle drops it back). Grind
            # matmuls on the ones4k row the moment its 8KB DMA lands. ----
            for w in range(16):
                pw = ps.tile([128, 512], f32, tag="a", name=f"pw_{w}")
                nc.tensor.matmul(
                    pw[:], lhsT=ones_sb[0:1, 0:128],
                    rhs=ones_sb[0:1, 0:512],
                    start=True, stop=True)

            # ---- Q projection for one (sc, p) group: 8 accumulating
            # matmuls + 2 DVE bias-drains into the half-tiles ----
            def emit_q(sc, p, tag):
                half, col = sc // 2, (sc % 2) * 512
                ps_t = ps.tile([128, 512], f32, tag=tag,
                               name=f"pq_{sc}_{p}")
                for dc in range(8):
                    nc.tensor.matmul(
                        ps_t[:],
                        lhsT=Wq_sb[:, dc, p * 128:(p + 1) * 128],
                        rhs=xts[sc][:, dc, :],
                        start=(dc == 0), stop=(dc == 7))
                bias = bqk_sb[:, p: p + 1]
                nc.vector.tensor_scalar_add(
                    Qt0_h[half][0:64, p, col:col + 512], ps_t[0:64, :],
                    bias[0:64, :])
                nc.vector.tensor_scalar_add(
                    Qt1_h[half][64:128, p, col:col + 512], ps_t[64:128, :],
                    bias[64:128, :])

            def emit_fill(n, label):
                # dummy matmuls: the PE p-state drops to 1.2GHz after ~3us
                # idle; grind through known DMA-wait windows (results unused)
                for w in range(n):
                    pf = ps.tile([128, 512], f32, tag="a",
                                 name=f"pf_{label}_{w}")
                    nc.tensor.matmul(
                        pf[:], lhsT=ones_sb[0:1, 0:128],
                        rhs=ones_sb[0:1, 0:512],
                        start=True, stop=True)

            emit_q(0, 0, "a")
            emit_q(0, 1, "ctx")
            emit_q(1, 0, "a")
            emit_q(1, 1, "ctx")

            with tc.tile_pool(name="vtp", bufs=2) as vtp, \
                 tc.tile_pool(name="expp", bufs=5) as expp, \
                 tc.tile_pool(name="ctxu", bufs=2) as ctxu, \
                 tc.tile_pool(name="outsb", bufs=3) as outsb:
                bcp = ctxu
                rscr = ctxu

                # ---- fused K+V projection: stationary packs the head's
                # 64 K-dims and 64 V-dims (halves swapped for odd heads so
                # every drain is partition-aligned), streaming xg ONCE.
                # K half -> Kt (transposed-K layout), V half -> VT
                # [vdim, keys], later PE-transposed into v4 [key, vdim].
                # VT drains go on ScalarE pre-attention (idle then) but on
                # DVE when injected (ScalarE runs the exps, and gpsimd
                # cannot read PSUM). ----
                VT_of = {}

                def emit_kv(hl, chunks, tags, vt_dve=False):
                    p, hp = hl // 2, hl % 2
                    kr = slice(hp * 64, hp * 64 + 64)       # K out rows
                    vr = slice(64 - hp * 64, 128 - hp * 64)  # V out rows
                    if hl not in VT_of:
                        VT_of[hl] = vtp.tile([128, SK], fp16, tag="vt",
                                             name=f"vt_{hl}")
                    VT = VT_of[hl]
                    bias = bqk_sb[:, 2 + p: 3 + p]
                    for (c0, nn), tg in zip(chunks, tags):
                        t_ = ps.tile([128, nn], f32, tag=tg,
                                     name=f"pkv_{hl}_{c0}")
                        for dc in range(8):
                            nc.tensor.matmul(
                                t_[:],
                                lhsT=Wkv_sb[:, dc, hl * 128:(hl + 1) * 128],
                                rhs=xg_t[hl][:, dc, c0:c0 + nn],
                                start=(dc == 0), stop=(dc == 7))
                        nc.vector.tensor_scalar_add(
                            Kt_p[p][kr, c0:c0 + nn], t_[kr, :], bias[kr, :])
                        if vt_dve:
                            nc.vector.tensor_scalar_add(
                                VT[vr, c0:c0 + nn], t_[vr, :],
                                bvT_sb[vr, hl:hl + 1])
                        else:
                            nc.scalar.add(VT[vr, c0:c0 + nn], t_[vr, :],
                                          bvT_sb[vr, hl:hl + 1])

                def emit_t(hl, groups, tags):
                    # transpose VT 128-key chunks into v4 [key, vdim],
                    # batched so one DVE drain covers several kt
                    hp = hl % 2
                    vr = slice(64 - hp * 64, 128 - hp * 64)
                    VT = VT_of[hl]
                    for (k0, kl), tg in zip(groups, tags):
                        pt_g = ps.tile([128, kl, 64], fp16, tag=tg,
                                       name=f"pt_{hl}_{k0}")
                        for j in range(kl):
                            nc.tensor.transpose(
                                pt_g[:, j, :],
                                VT[vr, (k0 + j) * 128:(k0 + j + 1) * 128],
                                id_sb[vr, :])
                        nc.vector.tensor_copy(
                            v4_h[hl][:, k0:k0 + kl, 0:64], pt_g[:])

                kh = KT // 2 + 1  # transpose group split (5/4 for KT=9)
                CH512 = []
                c0 = 0
                while c0 < SK:
                    CH512.append((c0, min(512, SK - c0)))
                    c0 += 512
                TGR = [(k0, kl) for k0, kl in ((0, kh), (kh, KT - kh))
                       if kl > 0]

                # pair-0 heads projected before attention; pair-1 heads are
                # injected into the early attention steps (see inj below).
                # Heads 0-2 (and all of Qproj) run BEFORE the attention
                # loop, emitted in DMA-arrival order; only head 3 (whose
                # xg lands last) is injected into the early attention
                # steps. Injecting more was measured strictly worse: each
                # injected PE unit stretches the exp stream by nearly its
                # full cost (the absorbable ScalarE slack before block 2
                # is only ~5us), while serial projections run gap-free.
                # Fillers bridge the xg DMA waits: a >2us PE stall drops
                # the p-state, and the steady attention stalls (~1us per
                # step) never give the 3us of continuous work needed to
                # re-promote -- entering attention cold costs ~20%.
                # T(hl) needs its head's VT bias-drain (ScalarE, ~1.5us
                # incl semaphores) -- cover that latency and the xg DMA
                # waits with real work (Qproj sc2/3) or fillers so the PE
                # never idles (a >2us stall demotes the p-state).
                emit_fill(8, "x0")
                emit_kv(0, CH512, ["a", "ctx"] * len(CH512))
                emit_q(2, 0, "a")
                emit_t(0, TGR, ["a", "ctx"])
                emit_q(2, 1, "ctx")
                emit_q(3, 0, "a")
                emit_q(3, 1, "ctx")
                emit_fill(4, "x1")
                emit_kv(1, CH512, ["ctx", "a"] * len(CH512))
                emit_fill(6, "t1")
                emit_t(1, TGR, ["a", "ctx"])
                emit_fill(4, "x2")
                emit_kv(2, CH512, ["a", "ctx"] * len(CH512))
                emit_fill(6, "t2")
                emit_t(2, TGR, ["a", "ctx"])

                # ---- attention: flat pipeline over 4 blocks ----
                # block b: p = b//2, half = b%2. Step t: block(t) = t//KT.
                NSTEP = 4 * KT

                def blk(t):
                    return (t // KT) // 2, (t // KT) % 2, t % KT

                def emit_scores(t):
                    p, half, kt = blk(t)
                    s0 = ps.tile([128, 1024], f32, tag="a",
                                 name=f"s0_{t}")
                    s1 = ps.tile([128, 1024], f32, tag="a",
                                 name=f"s1_{t}")
                    lhsT = Kt_p[p][:, kt * 128:(kt + 1) * 128]
                    for s_t, qsrc in ((s0, Qt0_h[half]), (s1, Qt1_h[half])):
                        for qc in range(2):
                            nc.tensor.matmul(
                                s_t[:, qc * 512:(qc + 1) * 512],
                                lhsT=lhsT,
                                rhs=qsrc[:, p, qc * 512:(qc + 1) * 512],
                                start=True, stop=True)
                    return s0, s1

                def emit_exp(t, sc_t):
                    p, half, kt = blk(t)
                    ets = []
                    for hp in range(2):
                        et = expp.tile([128, 1024], fp16, tag="et",
                                       name=f"et_{t}_{hp}")
                        nc.scalar.activation(
                            et[:], sc_t[hp][:], Exp,
                            bias=maskT_sb[:, kt * 4 + 2 * p + hp:
                                          kt * 4 + 2 * p + hp + 1],
                            scale=1.0)
                        ets.append(et)
                    return ets

                ctxs_of_block = {}

                def emit_ctx(t, ets):
                    p, half, kt = blk(t)
                    b = t // KT
                    if kt == 0:
                        ctxs_of_block[b] = (
                            ps.tile([65, 1024], f32, tag="ctx", name=f"c0_{b}"),
                            ps.tile([65, 1024], f32, tag="ctx", name=f"c1_{b}"))
                    ctxs = ctxs_of_block[b]
                    for hp in range(2):
                        for qc in range(2):
                            nc.tensor.matmul(
                                ctxs[hp][:, qc * 512:(qc + 1) * 512],
                                lhsT=v4_h[2 * p + hp][:, kt, :],
                                rhs=ets[hp][:, qc * 512:(qc + 1) * 512],
                                start=(kt == 0), stop=(kt == KT - 1))

                norm_q = []

                def emit_drain(b):
                    # block b finished accumulating: move ctx out of PSUM,
                    # compute 1/rowsum (on a DMA-transposed [128, 8] view --
                    # DVE op time scales with FREE size, so any [*, 1024]
                    # reciprocal would take ~7us), broadcast it across
                    # partitions, queue the normalize multiplies.
                    p, half = b // 2, b % 2
                    ctxs = ctxs_of_block.pop(b)
                    ctxUs, bcs = [], []
                    for hp in range(2):
                        ctxU = ctxu.tile([65, 1024], fp16, tag="cu",
                                         bufs=2, name=f"cu_{b}_{hp}")
                        nc.vector.tensor_copy(ctxU[:], ctxs[hp][:])
                        s128 = rscr.tile([128, 8], fp16, tag="sm",
                                         name=f"sm_{b}_{hp}")
                        nc.sync.dma_start(s128[:], ctxU[64:65, :])
                        r128 = rscr.tile([128, 8], fp16, tag="rc",
                                         name=f"rc_{b}_{hp}")
                        with nc.allow_low_precision(
                                reason="fp16 softmax-sum reciprocal"):
                            nc.vector.reciprocal(r128[:], s128[:])
                        rs_t = rscr.tile([1, 1024], fp16, tag="rs",
                                         name=f"rs_{b}_{hp}")
                        nc.sync.dma_start(rs_t[:], r128[:])
                        ctxUs.append(ctxU)
                        bc_t = bcp.tile([64, 1024], fp16, tag="bc",
                                        bufs=2, name=f"bc_{b}_{hp}")
                        nc.gpsimd.partition_broadcast(bc_t[:],
                                                      rs_t[0:1, :])
                        bcs.append(bc_t)

                    box = {}

                    def step(j):
                        hp_, qc = j // 2, j % 2
                        if hp_ == 0:
                            tgt = ctxT_hp[half][p][0:64,
                                                   qc * 512:(qc + 1) * 512]
                        else:
                            if "t" not in box:
                                box["t"] = ctxu.tile([64, 1024], fp16,
                                                     tag="cn", bufs=2,
                                                     name=f"cn_{b}")
                            tgt = box["t"][0:64, qc * 512:(qc + 1) * 512]
                        mul_rhs = bcs[hp_][0:64, qc * 512:(qc + 1) * 512]
                        nc.vector.tensor_mul(
                            tgt, ctxUs[hp_][0:64, qc * 512:(qc + 1) * 512],
                            mul_rhs)
                        if hp_ == 1 and qc == 1:
                            nc.sync.dma_start(
                                ctxT_hp[half][p][64:128, :], box["t"][0:64, :])

                    norm_q.extend([lambda j=j: step(j)
                                   for j in (0, 1, 2, 3)])

                emit_fill(4, "x3")
                emit_kv(3, CH512, ["ctx", "a"] * len(CH512))
                emit_fill(6, "t3")
                emit_t(3, TGR, ["a", "ctx"])
                inj = {}

                # output-projection unit: 4 matmuls + a PSUM->SBUF drain
                # copy (DVE for even qt, ScalarE for odd) into qt-pair
                # tiles; the pair's output DMA rides the sync queue. All po
                # tiles are tag "a" (the "ctx" slots stay coupled to block
                # 3's accumulators until its ScalarE copies run).
                ob_of = {}

                def emit_po(qt):
                    half, c = qt // 8, qt % 8
                    po = ps.tile([128, 1024], f32, tag="a", name=f"po_{qt}")
                    for p_ in range(2):
                        for ec in range(2):
                            nc.tensor.matmul(
                                po[:, ec * 512:(ec + 1) * 512],
                                lhsT=ctxT_hp[half][p_][:,
                                                       c * 128:(c + 1) * 128],
                                rhs=Wo_sb[:, p_, ec * 512:(ec + 1) * 512],
                                start=(p_ == 0), stop=(p_ == 1))
                    if qt % 2 == 0:
                        ob = outsb.tile([128, 2, 1024], fp16,
                                        tag="ob", name=f"ob_{qt}")
                        ob_of[qt // 2] = ob
                        nc.vector.tensor_copy(ob[:, 0, :], po[:])
                    else:
                        ob = ob_of[qt // 2]
                        nc.scalar.copy(ob[:, 1, :], po[:])
                        nc.sync.dma_start(
                            out_d[(qt - 1) * 128:(qt + 1) * 128, :].rearrange(
                                "(c p) e -> p c e", p=128),
                            ob[:])

                sc_cur = emit_scores(0)
                prev = None  # (t-1, ets)
                for t in range(NSTEP):
                    ets = emit_exp(t, sc_cur)
                    if prev is not None:
                        emit_ctx(prev[0], prev[1])
                        if prev[0] % KT == KT - 1:
                            emit_drain(prev[0] // KT)
                    if t < NSTEP - 1:
                        sc_cur = emit_scores(t + 1)
                    if t in inj:
                        inj[t]()
                    prev = (t, ets)
                    if norm_q and 2 <= (t % KT) <= 5:
                        norm_q.pop(0)()
                emit_ctx(prev[0], prev[1])

                # ---- tail: block 3's drain hand-interleaved with the
                # output projection. Everything after the last exp funnels
                # through the in-order DVE and ScalarE queues (only they
                # can read PSUM), so the emission order is the schedule:
                # chain pieces (ScalarE sum-rows -> DVE recip_approx ->
                # gpsimd broadcast -> DVE muls) woven between po/ob units
                # so no queue blocks behind another's dependency. ----
                ctxs3 = ctxs_of_block.pop(3)
                sr3, bc3, cu3, box3 = {}, {}, {}, {}
                for hp in (1, 0):
                    # ScalarE: sum rows, right after its last exp
                    sr3[hp] = rscr.tile([1, 1024], f32, tag="sr", bufs=1,
                                        name=f"sr3_{hp}")
                    nc.scalar.copy(sr3[hp][:], ctxs3[hp][64:65, :])

                def chain3(hp):
                    # DVE: 1/sums via recip_approx_fast on the [1,1024] f32
                    # row (~51 ULP; skips the two ~3us DMA-transpose hops
                    # of the mid-attention path); broadcast on gpsimd
                    rr = rscr.tile([1, 1024], f32, tag="rr", bufs=1,
                                   name=f"rr3_{hp}")
                    nc.vector.reciprocal_approx_fast(rr[:], sr3[hp][:])
                    rs_t = rscr.tile([1, 1024], fp16, tag="rs",
                                     name=f"rs3_{hp}")
                    nc.vector.tensor_copy(rs_t[:], rr[:])
                    bc_t = bcp.tile([64, 1024], fp16, tag="bc", bufs=2,
                                    name=f"bc3_{hp}")
                    nc.gpsimd.partition_broadcast(bc_t[:], rs_t[0:1, :])
                    bc3[hp] = bc_t

                def cu3_copy(hp):
                    # ScalarE (not DVE): DVE carries recips+muls+ob casts
                    cu3[hp] = ctxu.tile([65, 1024], fp16, tag="cu", bufs=2,
                                        name=f"cu3_{hp}")
                    nc.scalar.copy(cu3[hp][:], ctxs3[hp][:])

                def mul3(j):
                    hp_, qc = j // 2, j % 2
                    if hp_ == 0:
                        tgt = ctxT_hp[1][1][0:64, qc * 512:(qc + 1) * 512]
                    else:
                        if "t" not in box3:
                            box3["t"] = ctxu.tile([64, 1024], fp16,
                                                  tag="cn", bufs=2,
                                                  name="cn3")
                        tgt = box3["t"][0:64, qc * 512:(qc + 1) * 512]
                    nc.vector.tensor_mul(
                        tgt, cu3[hp_][0:64, qc * 512:(qc + 1) * 512],
                        bc3[hp_][0:64, qc * 512:(qc + 1) * 512])
                    if j == 3:
                        nc.sync.dma_start(ctxT_hp[1][1][64:128, :],
                                          box3["t"][0:64, :])

                emit_po(0)
                chain3(1)
                cu3_copy(1)
                emit_po(2)
                emit_po(1)
                chain3(0)
                cu3_copy(0)
                emit_po(3)
                mul3(2)
                mul3(3)
                emit_po(4)
                mul3(0)
                mul3(1)
                for qt in range(5, 8):
                    emit_po(qt)
                # keep the PE p-state at 2.4GHz through the chain
                emit_fill(10, "tail")
                for qt in range(8, 16):
                    emit_po(qt)

    nc.compile()
    return nc


def get_program(KT=9):
    key = ("nc", KT)
    if key not in _cache:
        _cache[key] = _build_program(KT)
    return _cache[key]


def make_in_maps(query, mask, W_qkv, b_qkv, W_out, b_out):
    query = np.asarray(query, dtype=np.float32)
    mask = np.asarray(mask)
    W_qkv = np.asarray(W_qkv, dtype=np.float32)
    b_qkv = np.asarray(b_qkv, dtype=np.float32)
    W_out = np.asarray(W_out, dtype=np.float32)
    bf = np.float16

    W3 = W_qkv.reshape(DIM, N_HEADS, DIM_PER_HEAD, 3)
    b3 = b_qkv.reshape(N_HEADS, DIM_PER_HEAD, 3)
    m2 = np.asarray(mask)[:, 0, :]  # [32, 2048] True = masked
    KT = max(1, int(np.ceil((~m2).sum(axis=1).max() / 128)))
    SK = KT * 128

    in_maps = []
    for c in range(N_CORES):
        b = c // 4
        h0 = (c % 4) * HEADS_PER_CORE
        hs = slice(h0, h0 + HEADS_PER_CORE)
        # weights/activations go to device in partition-major tile layout
        # [128, chunks, cols]: contiguous per partition -> cheap DMA
        # descriptor generation (see the dram_tensor comment)
        Wq_c = np.ascontiguousarray(
            (W3[:, hs, :, 0].reshape(DIM, 256) / SCALE)
            .reshape(8, 128, 256).transpose(1, 0, 2)).astype(bf)
        Wk_c = W3[:, hs, :, 1].reshape(DIM, 4, 64)
        Wv_c = W3[:, hs, :, 2].reshape(DIM, 4, 64)
        # fused K|V stationary: per head 128 cols, K half and V half swapped
        # for odd heads so every PSUM drain is partition-aligned
        Wkv_c = np.zeros((DIM, 4, 2, 64), dtype=np.float32)
        for hl in range(4):
            Wkv_c[:, hl, hl % 2, :] = Wk_c[:, hl, :]
            Wkv_c[:, hl, 1 - hl % 2, :] = Wv_c[:, hl, :]
        Wkv_c = np.ascontiguousarray(
            Wkv_c.reshape(8, 128, 512).transpose(1, 0, 2)).astype(bf)
        bq_c = (b3[hs, :, 0].reshape(256) / SCALE).astype(np.float32)
        bk_c = b3[hs, :, 1].reshape(256).astype(np.float32)
        bvT_c = np.ascontiguousarray(
            np.tile(b3[hs, :, 2].T, (2, 1))).astype(np.float32)  # [128, 4]
        id2_c = np.ascontiguousarray(
            np.concatenate([np.eye(64), np.eye(64)], axis=0)).astype(bf)
        bqk_c = np.ascontiguousarray(
            np.stack([bq_c[:128], bq_c[128:], bk_c[:128], bk_c[128:]], axis=1))
        Wo_c = np.ascontiguousarray(
            W_out[h0 * 64:(h0 + 4) * 64, :]
            .reshape(2, 128, 1024).transpose(1, 0, 2)).astype(bf)
        xT_c = np.ascontiguousarray(query[b].T).astype(bf)

        xg_c = np.zeros((4, DIM, SK), dtype=bf)
        maskT_c = np.zeros((128, 4 * KT), dtype=np.float32)
        for hl in range(4):
            bh = b * N_HEADS + h0 + hl
            idx = np.nonzero(~m2[bh])[0]
            n = len(idx)
            idx_pad = np.zeros(SK, dtype=np.int64)
            idx_pad[:n] = idx
            xg_c[hl] = xT_c[:, idx_pad]
            padded = np.arange(SK) >= n  # [SK] True = padding slot
            maskT_c[:, hl::4] = np.where(
                padded.reshape(KT, 128).T, np.float32(-30000.0),
                np.float32(0.0))
        # -> [4, 128, 8, 512]: query-chunk-major, partition-major within
        xT_t = np.ascontiguousarray(
            xT_c.reshape(8, 128, 4, 512).transpose(2, 1, 0, 3))
        # -> [4, 128, 8, SK]
        xg_t = np.ascontiguousarray(
            xg_c.reshape(4, 8, 128, SK).transpose(0, 2, 1, 3))
        in_maps.append({
            "xT": xT_t, "xg": xg_t, "Wq": Wq_c, "Wkv": Wkv_c,
            "Wo": Wo_c, "bqk": bqk_c, "bvT": bvT_c, "id2": id2_c,
            "maskT": maskT_c,
        })
    return in_maps, KT


def gather_outputs(results, b_out):
    b_out = np.asarray(b_out, dtype=np.float32)
    out = np.zeros((B, S, DIM), dtype=np.float32)
    for c in range(N_CORES):
        out[c // 4] += results[c]["out"].astype(np.float32)
    out += b_out[None, None, :]
    return out


def kernel(query, mask, W_qkv, b_qkv, W_out, b_out):
    from concourse.bass_utils import run_bass_kernel_spmd

    in_maps, KT = make_in_maps(query, mask, W_qkv, b_qkv, W_out, b_out)
    nc = get_program(KT)
    res = run_bass_kernel_spmd(nc, in_maps, list(range(N_CORES)))
    return gather_outputs(res.results, b_out)
